# revision 12
# baseline (speedup 1.0000x reference)
"""Trainium2 Bass kernel for nn_DGCNb (SDCN-style GNN + AE + NB head).

Strategy (8 NeuronCores, 1D node parallelism):
  - Row-shard the 8192 nodes: 1024 nodes per core. adj is passed pre-transposed
    per shard (adjT[c] = adj[rows_c, :].T, fp16 scaled by 8192), x pre-transposed
    (feature-major) and fp32r-pre-rounded.
  - Weights replicated, fp32r pre-rounded on host (fp32r matmuls run at full PE
    rate with ~2^-12 input rounding as the only error source).
  - gnn3 is re-associated: adj @ (H @ W) -> (adj @ H) @ W  (3.4x fewer FLOPs).
  - The sigma-blends (1-s)h + s*tra feeding a matmul are folded into stacked,
    pre-scaled weights; the blend feeding pass 3 is done explicitly.
  - Dead code skipped: AE decoder d1/d2/d3/x_bar is never used by any output.
  - Feature dims zero-padded to multiples of 128 (2000->2048, 500->512,
    1000->1024). Padding provably contributes exactly 0 to every output
    including the NB loss (after the host-side constant correction).
  - Per-GNN-layer AllGather of the small node-major T matrices (fp16).
  - NB loss (3 lgammas via 2-shift Stirling + log terms) computed on-chip in
    bf16 elementwise, reduced to per-partition partials; host finishes the mean.
"""
import numpy as np
import ml_dtypes

import concourse.bass as bass
import concourse.mybir as mybir
import concourse.tile as tile
from concourse import bacc
from concourse.bass_utils import run_bass_kernel_spmd
from concourse.masks import make_identity

F32 = mybir.dt.float32
F32R = mybir.dt.float32r
BF16 = mybir.dt.bfloat16
F16 = mybir.dt.float16
AF = mybir.ActivationFunctionType
OP = mybir.AluOpType
AX = mybir.AxisListType

NCORES = 8
NODES = 8192
SH = NODES // NCORES          # 1024 nodes per core
NIN, NINP = 2000, 2048        # genes (true, padded)
ET, E = 500, 512              # encoder width (true, padded)
NHT, NH = 1000, 1024          # fc1 hidden (true, padded)
NZ, NC = 10, 10
SIG = 0.3
ASCALE = 8192.0               # adj fp16 pre-scale
EPS = 1e-10
C_STIRLING = 0.5 * np.log(2.0 * np.pi)

KN = NINP // 128   # 16
KE = E // 128      # 4
KH = NH // 128     # 8
KA = NODES // 128  # 64
MN = SH // 128     # 8 node tiles per shard
G = NINP // 128    # 16 gene tiles
NT = 2             # 512-wide node halves


def _r32r(x):
    """Round fp32 array to fp32r (11-bit mantissa, RTN) on host."""
    x = np.ascontiguousarray(x, dtype=np.float32)
    u = x.view(np.uint32).astype(np.uint64)
    r = (u + 0x7FF + ((u >> 12) & 1)) & np.uint64(0xFFFFF000)
    return r.astype(np.uint32).view(np.float32)


def _pad2(a, r, c):
    a = np.asarray(a, np.float32)
    out = np.zeros((r, c), np.float32)
    out[:a.shape[0], :a.shape[1]] = a
    return out


_NC_CACHE = {}
STOP_AFTER = 99


def build():
    key = ("nc", STOP_AFTER)
    if key in _NC_CACHE:
        return _NC_CACHE[key]
    nc = bacc.Bacc("TRN2", target_bir_lowering=False, debug=False,
                   num_devices=NCORES)

    def din(name, shape, dt):
        return nc.dram_tensor(name, shape, dt, kind="ExternalInput")

    def dout(name, shape, dt):
        return nc.dram_tensor(name, shape, dt, kind="ExternalOutput")

    xT = din("xT", [NINP, SH], F32R)
    xTb = din("xTb", [NINP, SH], BF16)
    adjT = din("adjT", [NODES, SH], F16)
    w_enc1 = din("w_enc1", [NINP, E], F32R)
    w_enc2 = din("w_enc2", [E, E], F32R)
    w_enc3 = din("w_enc3", [E, NINP], F32R)
    w_z = din("w_z", [NINP, NZ], F32R)
    w_g1 = din("w_g1", [NINP, E], F32R)
    w_g2c = din("w_g2c", [2 * E, E], F32R)
    w_g3 = din("w_g3", [E, NINP], F32R)
    w_g4c = din("w_g4c", [2 * NINP, NZ], F32R)
    w_g5c = din("w_g5c", [2 * NZ, NZ], F32R)
    w_fc1 = din("w_fc1", [NZ, NH], F32R)
    w_disp = din("w_disp", [NH, NINP], F32R)
    w_mu = din("w_mu", [NH, NINP], F32R)
    b_enc1 = din("b_enc1", [128, KE], F32)
    b_enc2p = din("b_enc2p", [128, KE], F32)
    b_enc2r = din("b_enc2r", [1, E], F32)
    b_enc3p = din("b_enc3p", [128, KN], F32)
    b_z = din("b_z", [NZ, 1], F32)
    b_fc1p = din("b_fc1p", [128, KH], F32)
    b_dispp = din("b_dispp", [128, KN], F32)
    b_mup = din("b_mup", [128, KN], F32)
    cTm2 = din("cTm2", [NZ, NC], F32R)     # -2 * cluster.T
    cT = din("cT", [NZ, NC], F32)          # cluster.T

    o_tra1 = dout("o_tra1", [E, SH], F32)
    o_tra2 = dout("o_tra2", [SH, E], F32)
    o_tra3 = dout("o_tra3", [NINP, SH], F32)
    o_z = dout("o_z", [NZ, SH], F32)
    o_h = dout("o_h", [NZ, SH], F32)
    o_pred = dout("o_pred", [NZ, SH], F32)
    o_q = dout("o_q", [NC, SH], F32)
    o_out = dout("o_out", [NINP, SH], F32)
    o_loss = dout("o_loss", [128, 1], F32)

    with tile.TileContext(nc) as tc:
        with (
            tc.tile_pool(name="outer", bufs=1) as outer,
            tc.tile_pool(name="streams", bufs=4) as streams,
            tc.tile_pool(name="ps", bufs=8, space="PSUM") as psum,
            tc.tile_pool(name="dram", bufs=1, space="DRAM") as dram,
        ):
            _emit(nc, tc, outer, streams, psum, dram, locals())

    nc.finalize()
    _NC_CACHE[key] = nc
    return nc


def _emit(nc, tc, outer, streams, psum, dram, T):
    xT, xTb, adjT = T["xT"], T["xTb"], T["adjT"]
    w_enc1, w_enc2, w_enc3, w_z = T["w_enc1"], T["w_enc2"], T["w_enc3"], T["w_z"]
    w_g1, w_g2c, w_g3, w_g4c, w_g5c = (T["w_g1"], T["w_g2c"], T["w_g3"],
                                       T["w_g4c"], T["w_g5c"])
    w_fc1, w_disp, w_mu = T["w_fc1"], T["w_disp"], T["w_mu"]
    o_tra1, o_tra2, o_tra3 = T["o_tra1"], T["o_tra2"], T["o_tra3"]
    o_z, o_h, o_pred, o_q, o_out, o_loss = (T["o_z"], T["o_h"], T["o_pred"],
                                            T["o_q"], T["o_out"], T["o_loss"])

    def ps512(dt=F32):
        return psum.tile([128, 512], dt, tag="ps", name="ps")

    # ---- constants ----
    ones_r1x128 = outer.tile([1, 128], F32, tag="ones128")
    nc.vector.memset(ones_r1x128[:], 1.0)
    ones_c10 = outer.tile([NZ, 1], F32, tag="ones10")
    nc.vector.memset(ones_c10[:], 1.0)
    ones_r1x10 = outer.tile([1, NZ], F32, tag="onesr10")
    nc.vector.memset(ones_r1x10[:], 1.0)
    ones_r1x512 = outer.tile([1, 512], F32, tag="onesr512")
    nc.vector.memset(ones_r1x512[:], 1.0)
    ident = outer.tile([128, 128], F16, tag="ident")
    make_identity(nc, ident[:])
    cb2 = outer.tile([128, 1], BF16, tag="cb2")
    nc.vector.memset(cb2[:], 2.0)
    cbe = outer.tile([128, 1], BF16, tag="cbe")
    nc.vector.memset(cbe[:], EPS)

    # ---- small loads (biases etc.) ----
    def load_small(name, src, shape, dt=F32):
        t = outer.tile(shape, dt, tag=name)
        nc.sync.dma_start(t[:], src[tuple(slice(0, s) for s in shape)])
        return t

    bias1 = load_small("bias1", T["b_enc1"], [128, KE])
    bias2p = load_small("bias2p", T["b_enc2p"], [128, KE])
    bias2r = load_small("bias2r", T["b_enc2r"], [1, E])
    bias3p = load_small("bias3p", T["b_enc3p"], [128, KN])
    biasz = load_small("biasz", T["b_z"], [NZ, 1])
    biasf1 = load_small("biasf1", T["b_fc1p"], [128, KH])
    biasd = load_small("biasd", T["b_dispp"], [128, KN])
    biasm = load_small("biasm", T["b_mup"], [128, KN])
    ctm2 = load_small("ctm2", T["cTm2"], [NZ, NC], F32R)
    ctf = load_small("ctf", T["cT"], [NZ, NC])

    partials = outer.tile([128, G], F32, tag="partials")

    # small persistent activations (feature-major [10, 1024])
    zT = outer.tile([NZ, SH], F32, tag="zT")
    zr = outer.tile([NZ, SH], F32R, tag="zr")
    hT = outer.tile([NZ, SH], F32, tag="hT")
    relu_h = outer.tile([NZ, SH], F32R, tag="relu_h")
    h_r = outer.tile([NZ, SH], F32R, tag="h_r")
    h5T = outer.tile([NZ, SH], F32, tag="h5T")
    t45 = outer.tile([128, KA, NZ], F16, tag="t45full")

    # AG dram buffers
    ag_in = [dram.tile([SH, E], F16, name=f"agin{i}") for i in range(3)]
    ag_out = [dram.tile([NODES, E], F16, name=f"agout{i}") for i in range(3)]
    ag_in_s = [dram.tile([SH, NZ], F16, name=f"agins{i}") for i in range(2)]
    ag_out_s = [dram.tile([NODES, NZ], F16, name=f"agouts{i}") for i in range(2)]

    RG = [list(range(NCORES))]

    def allgather(src, dst):
        nc.gpsimd.collective_compute(
            "AllGather", OP.bypass, replica_groups=RG,
            ins=[src.opt()], outs=[dst.opt()])

    # =============== region L1: tra1 / tra2 / h1 ===============
    with tc.tile_pool(name="L1", bufs=1) as L1:
        tra1 = L1.tile([128, KE, SH], F32R, tag="tra1")
        tra2T = L1.tile([128, KE, SH], F32R, tag="tra2T")
        tra2n = L1.tile([128, MN, E], F32R, tag="tra2n")
        h1T = L1.tile([128, KE, SH], F32R, tag="h1T")

        # ---------- enc1 + gnn1, fused over the xT stream ----------
        with tc.tile_pool(name="L2", bufs=1) as L2:
            w1 = L2.tile([128, KN, E], F32R, tag="w1res")
            nc.sync.dma_start(
                w1[:], w_enc1.ap().rearrange("(k p) m -> p k m", p=128))
            t1n = L2.tile([128, MN, E], F16, tag="t1n")
            for h in range(NT):   # node half
                pse = [ps512() for _ in range(KE)]
                psg = [ps512() for _ in range(KE)]
                for k in range(KN):
                    xkt = streams.tile([128, 512], F32R, tag="xkt")
                    nc.sync.dma_start(
                        xkt[:], xT[k * 128:(k + 1) * 128,
                                   h * 512:(h + 1) * 512])
                    g1t = streams.tile([128, E], F32R, tag="wstream")
                    nc.sync.dma_start(g1t[:], w_g1[k * 128:(k + 1) * 128, :])
                    st, sp = (k == 0), (k == KN - 1)
                    for m in range(KE):
                        nc.tensor.matmul(pse[m][:],
                                         w1[:, k, m * 128:(m + 1) * 128],
                                         xkt[:], start=st, stop=sp)
                    for m in range(KE):
                        nc.tensor.matmul(psg[m][:],
                                         xkt[:, m * 128:(m + 1) * 128],
                                         g1t[:], start=st, stop=sp)
                for m in range(KE):
                    nc.scalar.activation(tra1[:, m, h * 512:(h + 1) * 512],
                                         pse[m][:], AF.Relu,
                                         bias=bias1[:, m:m + 1])
                    nc.sync.dma_start(
                        o_tra1[m * 128:(m + 1) * 128, h * 512:(h + 1) * 512],
                        tra1[:, m, h * 512:(h + 1) * 512].bitcast(F32))
                for m in range(KE):
                    nc.scalar.activation(t1n[:, h * KE + m, :], psg[m][:],
                                         AF.Copy)
            for m in range(MN):
                nc.sync.dma_start(ag_in[0][m * 128:(m + 1) * 128, :],
                                  t1n[:, m, :])
        allgather(ag_in[0], ag_out[0])
        if STOP_AFTER < 1:
            return

        # ---------- enc2 (both orientations) ----------
        with tc.tile_pool(name="L2b", bufs=1) as L2b:
            w2c = L2b.tile([128, KE, E], F32R, tag="w2c")
            nc.sync.dma_start(
                w2c[:], w_enc2.ap().rearrange("(k p) m -> p k m", p=128))
            for m in range(KE):
                for h in range(NT):
                    ps = ps512()
                    for k in range(KE):
                        nc.tensor.matmul(ps[:],
                                         w2c[:, k, m * 128:(m + 1) * 128],
                                         tra1[:, k, h * 512:(h + 1) * 512],
                                         start=(k == 0), stop=(k == KE - 1))
                    nc.scalar.activation(tra2T[:, m, h * 512:(h + 1) * 512],
                                         ps[:], AF.Relu,
                                         bias=bias2p[:, m:m + 1])
            psn = [ps512() for _ in range(MN)]
            for k in range(KE):
                e2t = streams.tile([128, E], F32R, tag="wstream")
                nc.sync.dma_start(e2t[:], w_enc2[k * 128:(k + 1) * 128, :])
                for m in range(MN):
                    nc.tensor.matmul(psn[m][:],
                                     tra1[:, k, m * 128:(m + 1) * 128],
                                     e2t[:], start=(k == 0), stop=False)
            for m in range(MN):
                nc.tensor.matmul(psn[m][:], ones_r1x128[:], bias2r[:],
                                 start=False, stop=True)
                nc.scalar.activation(tra2n[:, m, :], psn[m][:], AF.Relu)
                nc.sync.dma_start(o_tra2[m * 128:(m + 1) * 128, :],
                                  tra2n[:, m, :].bitcast(F32))

        if STOP_AFTER < 2:
            return
        # ---------- enc3 -> o_tra3 (spilled to DRAM), z ----------
        with tc.tile_pool(name="L2c", bufs=2) as L2c:
            for m in range(KN):
                w3c = L2c.tile([128, KE, 128], F32R, tag="w3c")
                nc.sync.dma_start(
                    w3c[:], w_enc3[:, m * 128:(m + 1) * 128]
                    .rearrange("(k p) m -> p k m", p=128))
                for h in range(NT):
                    ps = ps512()
                    for k in range(KE):
                        nc.tensor.matmul(ps[:], w3c[:, k, :],
                                         tra2T[:, k, h * 512:(h + 1) * 512],
                                         start=(k == 0), stop=(k == KE - 1))
                    ev = L2c.tile([128, 512], F32R, tag="t3ev")
                    nc.scalar.activation(ev[:], ps[:], AF.Relu,
                                         bias=bias3p[:, m:m + 1])
                    nc.sync.dma_start(
                        o_tra3[m * 128:(m + 1) * 128, h * 512:(h + 1) * 512],
                        ev[:].bitcast(F32))
            wzt = L2c.tile([128, KN, NZ], F32R, tag="wzt")
            nc.sync.dma_start(
                wzt[:], w_z.ap().rearrange("(k p) m -> p k m", p=128))
            for h in range(NT):
                ps = ps512()
                for k in range(KN):
                    t3t = streams.tile([128, 512], F32R, tag="t3r")
                    nc.sync.dma_start(
                        t3t[:], o_tra3[k * 128:(k + 1) * 128,
                                       h * 512:(h + 1) * 512].bitcast(F32R))
                    nc.tensor.matmul(ps[:NZ, :], wzt[:, k, :], t3t[:],
                                     start=(k == 0), stop=(k == KN - 1))
                nc.scalar.activation(zT[:, h * 512:(h + 1) * 512], ps[:NZ, :],
                                     AF.Identity, bias=biasz[:, 0:1])
                nc.scalar.activation(zr[:, h * 512:(h + 1) * 512], ps[:NZ, :],
                                     AF.Identity, bias=biasz[:, 0:1])
            nc.sync.dma_start(o_z[:, :], zT[:])

        if STOP_AFTER < 3:
            return
        # ---------- adj pass helper (feature-major output) ----------
        def adj_pass_fmajor(ag_src, out3d, relu):
            for h in range(NT):
                ps = [ps512() for _ in range(KE)]
                for ka in range(KA):
                    tk = streams.tile([128, E], F16, tag="Tk")
                    nc.sync.dma_start(tk[:],
                                      ag_src[ka * 128:(ka + 1) * 128, :])
                    ah = streams.tile([128, 512], F16, tag="adjk")
                    nc.sync.dma_start(
                        ah[:], adjT[ka * 128:(ka + 1) * 128,
                                    h * 512:(h + 1) * 512])
                    st, sp = (ka == 0), (ka == KA - 1)
                    for f in range(KE):
                        nc.tensor.matmul(ps[f][:],
                                         tk[:, f * 128:(f + 1) * 128],
                                         ah[:], start=st, stop=sp)
                for f in range(KE):
                    nc.scalar.activation(out3d[:, f, h * 512:(h + 1) * 512],
                                         ps[f][:],
                                         AF.Relu if relu else AF.Copy,
                                         scale=1.0 / ASCALE)

        # ---------- pass1: h1 = relu(adj @ T1) ----------
        adj_pass_fmajor(ag_out[0], h1T, relu=True)

        if STOP_AFTER < 4:
            return
        # ---------- T2 = [h1; tra1] @ g2c -> AG2 ----------
        with tc.tile_pool(name="L3a", bufs=1) as L3a:
            t2n = L3a.tile([128, MN, E], F16, tag="t2n")
            psn = [ps512() for _ in range(MN)]
            for k in range(2 * KE):
                g2t = streams.tile([128, E], F32R, tag="wstream")
                nc.sync.dma_start(g2t[:], w_g2c[k * 128:(k + 1) * 128, :])
                src = h1T if k < KE else tra1
                kk = k % KE
                for m in range(MN):
                    nc.tensor.matmul(psn[m][:],
                                     src[:, kk, m * 128:(m + 1) * 128],
                                     g2t[:], start=(k == 0),
                                     stop=(k == 2 * KE - 1))
            for m in range(MN):
                nc.scalar.activation(t2n[:, m, :], psn[m][:], AF.Copy)
                nc.sync.dma_start(ag_in[1][m * 128:(m + 1) * 128, :],
                                  t2n[:, m, :])
        allgather(ag_in[1], ag_out[1])

        if STOP_AFTER < 5:
            return
        # ---------- pass2 (node-major) + H3in + AG3 ----------
        with tc.tile_pool(name="L3", bufs=1) as L3:
            h2n = L3.tile([128, MN, E], F32R, tag="h2n")
            for mh in range(2):
                ps = [ps512() for _ in range(4)]
                for ka in range(KA):
                    tk = streams.tile([128, E], F16, tag="Tk")
                    nc.sync.dma_start(tk[:],
                                      ag_out[1][ka * 128:(ka + 1) * 128, :])
                    ah = streams.tile([128, 512], F16, tag="adjk")
                    nc.sync.dma_start(
                        ah[:], adjT[ka * 128:(ka + 1) * 128,
                                    mh * 512:(mh + 1) * 512])
                    st, sp = (ka == 0), (ka == KA - 1)
                    for m in range(4):
                        nc.tensor.matmul(ps[m][:],
                                         ah[:, m * 128:(m + 1) * 128],
                                         tk[:], start=st, stop=sp)
                for m in range(4):
                    nc.scalar.activation(h2n[:, mh * 4 + m, :], ps[m][:],
                                         AF.Relu, scale=1.0 / ASCALE)
            with tc.tile_pool(name="L3b", bufs=2) as L3b:
                for m in range(MN):
                    tmp = L3b.tile([128, E], F32, tag="h3tmp")
                    nc.vector.tensor_scalar_mul(
                        tmp[:], tra2n[:, m, :].bitcast(F32), SIG)
                    h3i = L3b.tile([128, E], F16, tag="h3i")
                    nc.vector.scalar_tensor_tensor(
                        h3i[:], h2n[:, m, :].bitcast(F32), 1.0 - SIG, tmp[:],
                        op0=OP.mult, op1=OP.add)
                    nc.sync.dma_start(ag_in[2][m * 128:(m + 1) * 128, :],
                                      h3i[:])
        allgather(ag_in[2], ag_out[2])

    if STOP_AFTER < 6:
        return
    # =============== pass3 + gnn3 feature + T4 + AG4 ===============
    with tc.tile_pool(name="L4a", bufs=1) as L4a:
        s3T = L4a.tile([128, KE, SH], F32R, tag="s3T")

        def adj_pass_fmajor2(ag_src, out3d, relu):
            for h in range(NT):
                ps = [ps512() for _ in range(KE)]
                for ka in range(KA):
                    tk = streams.tile([128, E], F16, tag="Tk")
                    nc.sync.dma_start(tk[:],
                                      ag_src[ka * 128:(ka + 1) * 128, :])
                    ah = streams.tile([128, 512], F16, tag="adjk")
                    nc.sync.dma_start(
                        ah[:], adjT[ka * 128:(ka + 1) * 128,
                                    h * 512:(h + 1) * 512])
                    st, sp = (ka == 0), (ka == KA - 1)
                    for f in range(KE):
                        nc.tensor.matmul(ps[f][:],
                                         tk[:, f * 128:(f + 1) * 128],
                                         ah[:], start=st, stop=sp)
                for f in range(KE):
                    nc.scalar.activation(out3d[:, f, h * 512:(h + 1) * 512],
                                         ps[f][:],
                                         AF.Relu if relu else AF.Copy,
                                         scale=1.0 / ASCALE)

        adj_pass_fmajor2(ag_out[2], s3T, relu=False)
        h3T = L4a.tile([128, KN, SH], F32R, tag="h3T")
        with tc.tile_pool(name="L4w", bufs=2) as L4w:
            for m in range(KN):
                w3t = L4w.tile([128, KE, 128], F32R, tag="wg3c")
                nc.sync.dma_start(
                    w3t[:], w_g3[:, m * 128:(m + 1) * 128]
                    .rearrange("(k p) m -> p k m", p=128))
                for h in range(NT):
                    ps = ps512()
                    for k in range(KE):
                        nc.tensor.matmul(ps[:], w3t[:, k, :],
                                         s3T[:, k, h * 512:(h + 1) * 512],
                                         start=(k == 0), stop=(k == KE - 1))
                    nc.scalar.activation(h3T[:, m, h * 512:(h + 1) * 512],
                                         ps[:], AF.Relu)
        # T4^T [10, 1024] = g4c.T @ [h3T; tra3]
        with tc.tile_pool(name="L4t", bufs=1) as L4t:
            w4t = L4t.tile([128, 2 * KN, NZ], F32R, tag="w4t")
            nc.sync.dma_start(
                w4t[:], w_g4c.ap().rearrange("(k p) m -> p k m", p=128))
            t4T = L4t.tile([NZ, SH], F16, tag="t4T")
            for h in range(NT):
                ps = ps512()
                for k in range(2 * KN):
                    if k < KN:
                        rhs = h3T[:, k, h * 512:(h + 1) * 512]
                    else:
                        t3t = streams.tile([128, 512], F32R, tag="t3r")
                        nc.sync.dma_start(
                            t3t[:],
                            o_tra3[(k - KN) * 128:(k - KN + 1) * 128,
                                   h * 512:(h + 1) * 512].bitcast(F32R))
                        rhs = t3t[:]
                    nc.tensor.matmul(ps[:NZ, :], w4t[:, k, :], rhs,
                                     start=(k == 0), stop=(k == 2 * KN - 1))
                nc.scalar.activation(t4T[:, h * 512:(h + 1) * 512],
                                     ps[:NZ, :], AF.Copy)
            t4n = L4t.tile([128, MN, NZ], F16, tag="t4n")
            for j in range(MN):
                pst = ps512(F16)
                nc.tensor.transpose(pst[:, :NZ],
                                    t4T[:, j * 128:(j + 1) * 128],
                                    ident[:NZ, :NZ])
                nc.scalar.activation(t4n[:, j, :], pst[:, :NZ], AF.Copy)
                nc.sync.dma_start(ag_in_s[0][j * 128:(j + 1) * 128, :],
                                  t4n[:, j, :])
    allgather(ag_in_s[0], ag_out_s[0])

    if STOP_AFTER < 7:
        return
    # =============== pass4: h = adj @ T4 (no relu on h) ===============
    nc.sync.dma_start(
        t45[:], ag_out_s[0].rearrange("(k p) m -> p k m", p=128))
    ps0, ps1 = ps512(), ps512()
    for ka in range(KA):
        ak = streams.tile([128, SH], F16, tag="adjw")
        nc.sync.dma_start(ak[:], adjT[ka * 128:(ka + 1) * 128, :])
        st, sp = (ka == 0), (ka == KA - 1)
        nc.tensor.matmul(ps0[:NZ, :], t45[:, ka, :], ak[:, :512],
                         start=st, stop=sp)
        nc.tensor.matmul(ps1[:NZ, :], t45[:, ka, :], ak[:, 512:],
                         start=st, stop=sp)
    for h, ps in ((0, ps0), (1, ps1)):
        sl = slice(h * 512, (h + 1) * 512)
        nc.scalar.activation(hT[:, sl], ps[:NZ, :], AF.Copy, scale=1.0 / ASCALE)
        nc.scalar.activation(relu_h[:, sl], ps[:NZ, :], AF.Relu,
                             scale=1.0 / ASCALE)
        nc.scalar.activation(h_r[:, sl], ps[:NZ, :], AF.Copy,
                             scale=1.0 / ASCALE)
    nc.sync.dma_start(o_h[:, :], hT[:])

    if STOP_AFTER < 8:
        return
    # =============== T5 + AG5 + pass5 ===============
    with tc.tile_pool(name="L5", bufs=1) as L5:
        w5a = L5.tile([NZ, NZ], F32R, tag="w5a")
        nc.sync.dma_start(w5a[:], w_g5c[0:NZ, :])
        w5b = L5.tile([NZ, NZ], F32R, tag="w5b")
        nc.sync.dma_start(w5b[:], w_g5c[NZ:2 * NZ, :])
        t5T = L5.tile([NZ, SH], F16, tag="t5T")
        for h in range(NT):
            sl = slice(h * 512, (h + 1) * 512)
            ps = ps512()
            nc.tensor.matmul(ps[:NZ, :], w5a[:], relu_h[:, sl],
                             start=True, stop=False)
            nc.tensor.matmul(ps[:NZ, :], w5b[:], zr[:, sl],
                             start=False, stop=True)
            nc.scalar.activation(t5T[:, h * 512:(h + 1) * 512],
                                 ps[:NZ, :], AF.Copy)
        t5n = L5.tile([128, MN, NZ], F16, tag="t5n")
        for j in range(MN):
            pst = ps512(F16)
            nc.tensor.transpose(pst[:, :NZ], t5T[:, j * 128:(j + 1) * 128],
                                ident[:NZ, :NZ])
            nc.scalar.activation(t5n[:, j, :], pst[:, :NZ], AF.Copy)
            nc.sync.dma_start(ag_in_s[1][j * 128:(j + 1) * 128, :],
                              t5n[:, j, :])
    allgather(ag_in_s[1], ag_out_s[1])
    nc.sync.dma_start(
        t45[:], ag_out_s[1].rearrange("(k p) m -> p k m", p=128))
    ps0, ps1 = ps512(), ps512()
    for ka in range(KA):
        ak = streams.tile([128, SH], F16, tag="adjw")
        nc.sync.dma_start(ak[:], adjT[ka * 128:(ka + 1) * 128, :])
        st, sp = (ka == 0), (ka == KA - 1)
        nc.tensor.matmul(ps0[:NZ, :], t45[:, ka, :], ak[:, :512],
                         start=st, stop=sp)
        nc.tensor.matmul(ps1[:NZ, :], t45[:, ka, :], ak[:, 512:],
                         start=st, stop=sp)
    for h, ps in ((0, ps0), (1, ps1)):
        nc.scalar.activation(h5T[:, h * 512:(h + 1) * 512], ps[:NZ, :],
                             AF.Copy, scale=1.0 / ASCALE)

    if STOP_AFTER < 9:
        return
    # =============== predict = softmax(h5), q (student-t) ===============
    with tc.tile_pool(name="L6", bufs=1) as L6:
        def rowsum_bcast_mult(src, dst):
            sm = L6.tile([1, SH], F32, tag="sm")
            for h in range(NT):
                ps = ps512()
                nc.tensor.matmul(ps[:1, :], ones_c10[:],
                                 src[:, h * 512:(h + 1) * 512],
                                 start=True, stop=True)
                nc.scalar.activation(sm[:, h * 512:(h + 1) * 512],
                                     ps[:1, :], AF.Copy)
            rs = L6.tile([1, SH], F32, tag="rs")
            nc.vector.reciprocal(rs[:], sm[:])
            bc = L6.tile([NZ, SH], F32, tag="bc")
            for h in range(NT):
                ps = ps512()
                nc.tensor.matmul(ps[:NZ, :], ones_r1x10[:],
                                 rs[:, h * 512:(h + 1) * 512],
                                 start=True, stop=True)
                nc.scalar.activation(bc[:, h * 512:(h + 1) * 512],
                                     ps[:NZ, :], AF.Copy)
            nc.vector.tensor_tensor(dst[:], src[:], bc[:], op=OP.mult)

        ex = L6.tile([NZ, SH], F32, tag="ex")
        nc.scalar.activation(ex[:], h5T[:], AF.Exp)
        pred = L6.tile([NZ, SH], F32, tag="pred")
        rowsum_bcast_mult(ex, pred)
        nc.sync.dma_start(o_pred[:, :], pred[:])

        hsq = L6.tile([NZ, SH], F32, tag="hsq")
        nc.scalar.activation(hsq[:], hT[:], AF.Square)
        hn2 = L6.tile([1, SH], F32, tag="hn2")
        for h in range(NT):
            ps = ps512()
            nc.tensor.matmul(ps[:1, :], ones_c10[:],
                             hsq[:, h * 512:(h + 1) * 512],
                             start=True, stop=True)
            nc.scalar.activation(hn2[:, h * 512:(h + 1) * 512],
                                 ps[:1, :], AF.Copy)
        csq = L6.tile([NZ, NC], F32, tag="csq")
        nc.scalar.activation(csq[:], ctf[:], AF.Square)
        pcs = ps512()
        nc.tensor.matmul(pcs[:1, :NC], ones_c10[:], csq[:],
                         start=True, stop=True)
        cn2 = L6.tile([1, NC], F32, tag="cn2")
        nc.scalar.activation(cn2[:], pcs[:1, :NC], AF.Copy)
        qu = L6.tile([NC, SH], F32, tag="qu")
        for h in range(NT):
            sl = slice(h * 512, (h + 1) * 512)
            ps = ps512()
            nc.tensor.matmul(ps[:NC, :], ctm2[:], h_r[:, sl],
                             start=True, stop=False)
            nc.tensor.matmul(ps[:NC, :], cn2[:], ones_r1x512[:],
                             start=False, stop=False)
            nc.tensor.matmul(ps[:NC, :], ones_r1x10[:], hn2[:, sl],
                             start=False, stop=True)
            dpo = L6.tile([NC, 512], F32, tag="dpo")
            nc.scalar.activation(dpo[:], ps[:NC, :], AF.Copy, bias=1.0)
            nc.vector.reciprocal(qu[:, sl], dpo[:])
        qn = L6.tile([NC, SH], F32, tag="qn")
        rowsum_bcast_mult(qu, qn)
        nc.sync.dma_start(o_q[:, :], qn[:])

    if STOP_AFTER < 10:
        return
    # =============== NB head + loss ===============
    with tc.tile_pool(name="L7", bufs=1) as L7:
        deco = L7.tile([128, KH, SH], F32R, tag="deco")
        wf = L7.tile([NZ, NH], F32R, tag="wf")
        nc.sync.dma_start(wf[:], w_fc1[:, :])
        for m in range(KH):
            for h in range(NT):
                ps = ps512()
                nc.tensor.matmul(ps[:], wf[:, m * 128:(m + 1) * 128],
                                 relu_h[:, h * 512:(h + 1) * 512],
                                 start=True, stop=True)
                nc.scalar.activation(deco[:, m, h * 512:(h + 1) * 512],
                                     ps[:], AF.Relu, bias=biasf1[:, m:m + 1])

        if STOP_AFTER < 11:
            return
        with (
            tc.tile_pool(name="L7b", bufs=2) as L7b,
            tc.tile_pool(name="L7c", bufs=1) as L7c,
        ):

            def bt2(name):
                return L7b.tile([128, SH], BF16, tag=name, name=name)

            def bt1(name):
                return L7c.tile([128, SH], BF16, tag=name, name=name)

            def lgamma_approx(z, out):
                """L'(z) = (z+1.5)ln(z+2) - ln(z(z+1)) + 1/(12(z+2))."""
                p2 = bt1("p2")
                nc.vector.scalar_tensor_tensor(p2[:], z, 1.0, z,
                                               op0=OP.add, op1=OP.mult)
                lnw = bt1("lnw")
                nc.scalar.activation(lnw[:], z, AF.Ln, bias=cb2[:, 0:1])
                lnp = bt1("lnp")
                nc.scalar.activation(lnp[:], p2[:], AF.Ln)
                rw = bt1("rw")
                nc.scalar.activation(rw[:], lnw[:], AF.Exp, scale=-1.0)
                nc.vector.scalar_tensor_tensor(out, z, 1.5, lnw[:],
                                               op0=OP.add, op1=OP.mult)
                nc.vector.tensor_tensor(out, out, lnp[:], op=OP.subtract)
                nc.vector.scalar_tensor_tensor(out, rw[:], 1.0 / 12.0, out,
                                               op0=OP.mult, op1=OP.add)

            for g in range(G):
                wdc = L7b.tile([128, KH, 128], F32R, tag="wdc")
                nc.sync.dma_start(
                    wdc[:], w_disp[:, g * 128:(g + 1) * 128]
                    .rearrange("(k p) m -> p k m", p=128))
                wmc = L7b.tile([128, KH, 128], F32R, tag="wmc")
                nc.sync.dma_start(
                    wmc[:], w_mu[:, g * 128:(g + 1) * 128]
                    .rearrange("(k p) m -> p k m", p=128))
                th = bt2("th")
                yh = bt2("yh")
                for h in range(NT):
                    sl = slice(h * 512, (h + 1) * 512)
                    psd = ps512()
                    for k in range(KH):
                        nc.tensor.matmul(psd[:], wdc[:, k, :],
                                         deco[:, k, sl],
                                         start=(k == 0), stop=(k == KH - 1))
                    eu = L7b.tile([128, 512], BF16, tag="eu")
                    nc.scalar.activation(eu[:], psd[:], AF.Exp,
                                         bias=biasd[:, g:g + 1])
                    nc.scalar.activation(th[:, sl], eu[:], AF.Ln, bias=1.0)
                    psm = ps512()
                    for k in range(KH):
                        nc.tensor.matmul(psm[:], wmc[:, k, :],
                                         deco[:, k, sl],
                                         start=(k == 0), stop=(k == KH - 1))
                    ot = L7b.tile([128, 512], F32, tag="ot")
                    nc.scalar.activation(ot[:], psm[:], AF.Relu,
                                         bias=biasm[:, g:g + 1])
                    nc.sync.dma_start(o_out[g * 128:(g + 1) * 128, sl], ot[:])
                    nc.scalar.activation(yh[:, sl], psm[:], AF.Relu,
                                         bias=biasm[:, g:g + 1])
                if STOP_AFTER < 12:
                    continue
                # ---- loss terms for this gene tile ----
                yb = bt2("yb")
                nc.sync.dma_start(yb[:], xTb[g * 128:(g + 1) * 128, :])
                la = bt1("la")
                lgamma_approx(th[:], la[:])
                b1 = bt1("b1")
                nc.vector.tensor_scalar_add(b1[:], yb[:], 1.0)
                lb = bt1("lb")
                lgamma_approx(b1[:], lb[:])
                cz = bt1("cz")
                nc.vector.tensor_tensor(cz[:], yb[:], th[:], op=OP.add)
                lc = bt1("lc")
                lgamma_approx(cz[:], lc[:])
                d1 = bt1("d1")
                nc.vector.tensor_tensor(d1[:], th[:], yh[:], op=OP.add)
                ln1 = bt1("ln1")
                nc.scalar.activation(ln1[:], d1[:], AF.Ln)
                lnth = bt1("lnth")
                nc.scalar.activation(lnth[:], th[:], AF.Ln)
                lnyh = bt1("lnyh")
                nc.scalar.activation(lnyh[:], yh[:], AF.Ln, bias=cbe[:, 0:1])
                m1 = bt1("m1")
                nc.vector.tensor_tensor(m1[:], th[:], yb[:], op=OP.add)
                nc.vector.tensor_tensor(m1[:], m1[:], ln1[:], op=OP.mult)
                m2 = bt1("m2")
                nc.vector.tensor_tensor(m2[:], th[:], lnth[:], op=OP.mult)
                m3 = bt1("m3")
                nc.vector.tensor_tensor(m3[:], yb[:], lnyh[:], op=OP.mult)
                nc.vector.tensor_tensor(m1[:], m1[:], m2[:], op=OP.subtract)
                nc.vector.tensor_tensor(m1[:], m1[:], m3[:], op=OP.subtract)
                nc.vector.tensor_tensor(la[:], la[:], lb[:], op=OP.add)
                nc.vector.tensor_tensor(la[:], la[:], lc[:], op=OP.subtract)
                nc.vector.tensor_tensor(la[:], la[:], m1[:], op=OP.add)
                nc.vector.tensor_reduce(partials[:, g:g + 1], la[:],
                                        axis=AX.X, op=OP.add)

    if STOP_AFTER < 12:
        return
    rsum = outer.tile([128, 1], F32, tag="rsum")
    nc.vector.tensor_reduce(rsum[:], partials[:], axis=AX.X, op=OP.add)
    nc.sync.dma_start(o_loss[:, :], rsum[:])


def _prep_inputs(x, adj, p):
    """Host-side sharding / layout prep. Returns per-core input maps."""
    x = np.asarray(x, np.float32)
    adj = np.asarray(adj, np.float32)

    def pad_r(w, r, c):
        return _r32r(_pad2(w, r, c))

    shared = {
        "w_enc1": pad_r(p["enc1_w"], NINP, E),
        "w_enc2": pad_r(p["enc2_w"], E, E),
        "w_enc3": pad_r(p["enc3_w"], E, NINP),
        "w_z": pad_r(p["z_w"], NINP, NZ),
        "w_g1": pad_r(p["gnn1_w"], NINP, E),
        "w_g2c": _r32r(np.concatenate([
            _pad2((1 - SIG) * np.asarray(p["gnn2_w"]), E, E),
            _pad2(SIG * np.asarray(p["gnn2_w"]), E, E)])),
        "w_g3": pad_r(p["gnn3_w"], E, NINP),
        "w_g4c": _r32r(np.concatenate([
            _pad2((1 - SIG) * np.asarray(p["gnn4_w"]), NINP, NZ),
            _pad2(SIG * np.asarray(p["gnn4_w"]), NINP, NZ)])),
        "w_g5c": _r32r(np.concatenate([
            (1 - SIG) * np.asarray(p["gnn5_w"], np.float32),
            SIG * np.asarray(p["gnn5_w"], np.float32)])),
        "w_fc1": pad_r(p["fc1_w"], NZ, NH),
        "w_disp": pad_r(p["disp_w"], NH, NINP),
        "w_mu": pad_r(p["mu_w"], NH, NINP),
        "cTm2": _r32r(np.ascontiguousarray(
            -2.0 * np.asarray(p["cluster"], np.float32).T)),
        "cT": np.ascontiguousarray(np.asarray(p["cluster"], np.float32).T),
    }

    def bias_p(b, padded, kt):
        v = np.zeros(padded, np.float32)
        b = np.asarray(b, np.float32)
        v[:b.shape[0]] = b
        return np.ascontiguousarray(v.reshape(kt, 128).T)

    shared["b_enc1"] = bias_p(p["enc1_b"], E, KE)
    shared["b_enc2p"] = bias_p(p["enc2_b"], E, KE)
    shared["b_enc2r"] = _pad2(
        np.asarray(p["enc2_b"], np.float32)[None, :], 1, E)
    shared["b_enc3p"] = bias_p(p["enc3_b"], NINP, KN)
    shared["b_z"] = np.ascontiguousarray(
        np.asarray(p["z_b"], np.float32).reshape(NZ, 1))
    shared["b_fc1p"] = bias_p(p["fc1_b"], NH, KH)
    shared["b_dispp"] = bias_p(p["disp_b"], NINP, KN)
    shared["b_mup"] = bias_p(p["mu_b"], NINP, KN)

    xp = np.zeros((NODES, NINP), np.float32)
    xp[:, :NIN] = x
    in_maps = []
    for c in range(NCORES):
        rows = slice(c * SH, (c + 1) * SH)
        xTc = np.ascontiguousarray(xp[rows].T)
        m = dict(shared)
        m["xT"] = _r32r(xTc)
        m["xTb"] = xTc.astype(ml_dtypes.bfloat16)
        m["adjT"] = np.ascontiguousarray(
            (adj[rows] * ASCALE).T.astype(np.float16))
        in_maps.append(m)
    return in_maps


def run_device(x, adj, params, trace=False):
    nc = build()
    in_maps = _prep_inputs(x, adj, params)
    return run_bass_kernel_spmd(nc, in_maps, list(range(NCORES)), trace=trace)


def assemble(results):
    """Gather per-core outputs into full reference-shaped arrays."""
    def cat_T(name, cols):  # feature-major [F, SH] -> [8192, cols]
        return np.ascontiguousarray(np.concatenate(
            [r[name].T for r in results], axis=0)[:, :cols]).astype(np.float32)

    output = cat_T("o_out", NIN)
    tra1 = cat_T("o_tra1", ET)
    tra3 = cat_T("o_tra3", NIN)
    tra2 = np.ascontiguousarray(np.concatenate(
        [r["o_tra2"] for r in results], axis=0)[:, :ET])
    z = cat_T("o_z", NZ)
    h = cat_T("o_h", NZ)
    predict = cat_T("o_pred", NZ)
    q = cat_T("o_q", NC)
    total = sum(float(r["o_loss"].sum()) for r in results)
    n_tot = NCORES * NINP * SH
    n_real = NODES * NIN
    loss = np.float32((total + n_tot * (C_STIRLING - 3.0)) / n_real)
    return (output, q, predict, z, np.float32(0.0), loss,
            (h, tra1, tra2, tra3))


def kernel(x, adj, params):
    res = run_device(x, adj, params, trace=False)
    return assemble(res.results)


# revision 13
# speedup vs baseline: 1.1782x; 1.1782x over previous
"""Trainium2 Bass kernel for nn_DGCNb (SDCN-style GNN + AE + NB head).

Strategy (8 NeuronCores, 1D node parallelism):
  - Row-shard the 8192 nodes: 1024 nodes per core. adj is passed pre-transposed
    per shard (adjT[c] = adj[rows_c, :].T, fp16 scaled by 8192), x pre-transposed
    (feature-major) and fp32r-pre-rounded.
  - Weights replicated, fp32r pre-rounded on host (fp32r matmuls run at full PE
    rate with ~2^-12 input rounding as the only error source).
  - gnn3 is re-associated: adj @ (H @ W) -> (adj @ H) @ W  (3.4x fewer FLOPs).
  - The sigma-blends (1-s)h + s*tra feeding a matmul are folded into stacked,
    pre-scaled weights; the blend feeding pass 3 is done explicitly.
  - Dead code skipped: AE decoder d1/d2/d3/x_bar is never used by any output.
  - Feature dims zero-padded to multiples of 128 (2000->2048, 500->512,
    1000->1024). Padding provably contributes exactly 0 to every output
    including the NB loss (after the host-side constant correction).
  - Per-GNN-layer AllGather of the small node-major T matrices (fp16).
  - NB loss (3 lgammas via 2-shift Stirling + log terms) computed on-chip in
    bf16 elementwise, reduced to per-partition partials; host finishes the mean.
"""
import numpy as np
import ml_dtypes

import concourse.bass as bass
import concourse.mybir as mybir
import concourse.tile as tile
from concourse import bacc
from concourse.bass_utils import run_bass_kernel_spmd
from concourse.masks import make_identity

F32 = mybir.dt.float32
F32R = mybir.dt.float32r
BF16 = mybir.dt.bfloat16
F16 = mybir.dt.float16
AF = mybir.ActivationFunctionType
OP = mybir.AluOpType
AX = mybir.AxisListType

NCORES = 8
NODES = 8192
SH = NODES // NCORES          # 1024 nodes per core
NIN, NINP = 2000, 2048        # genes (true, padded)
ET, E = 500, 512              # encoder width (true, padded)
NHT, NH = 1000, 1024          # fc1 hidden (true, padded)
NZ, NC = 10, 10
SIG = 0.3
ASCALE = 8192.0               # adj fp16 pre-scale
EPS = 1e-10
C_STIRLING = 0.5 * np.log(2.0 * np.pi)

KN = NINP // 128   # 16
KE = E // 128      # 4
KH = NH // 128     # 8
KA = NODES // 128  # 64
MN = SH // 128     # 8 node tiles per shard
G = NINP // 128    # 16 gene tiles
NT = 2             # 512-wide node halves


def _r32r(x):
    """Round fp32 array to fp32r (11-bit mantissa, RTN) on host."""
    x = np.ascontiguousarray(x, dtype=np.float32)
    u = x.view(np.uint32).astype(np.uint64)
    r = (u + 0x7FF + ((u >> 12) & 1)) & np.uint64(0xFFFFF000)
    return r.astype(np.uint32).view(np.float32)


def _pad2(a, r, c):
    a = np.asarray(a, np.float32)
    out = np.zeros((r, c), np.float32)
    out[:a.shape[0], :a.shape[1]] = a
    return out


_NC_CACHE = {}
STOP_AFTER = 99


def build():
    key = ("nc", STOP_AFTER)
    if key in _NC_CACHE:
        return _NC_CACHE[key]
    nc = bacc.Bacc("TRN2", target_bir_lowering=False, debug=False,
                   num_devices=NCORES)

    def din(name, shape, dt):
        return nc.dram_tensor(name, shape, dt, kind="ExternalInput")

    def dout(name, shape, dt):
        return nc.dram_tensor(name, shape, dt, kind="ExternalOutput")

    xT = din("xT", [NINP, SH], F32R)
    xTb = din("xTb", [NINP, SH], BF16)
    adjT = din("adjT", [NODES, SH], F16)
    w_enc1 = din("w_enc1", [NINP, E], F32R)
    w_enc2 = din("w_enc2", [E, E], F32R)
    w_enc3 = din("w_enc3", [E, NINP], F32R)
    w_z = din("w_z", [NINP, NZ], F32R)
    w_g1 = din("w_g1", [NINP, E], F32R)
    w_g2c = din("w_g2c", [2 * E, E], F32R)
    w_g3 = din("w_g3", [E, NINP], F32R)
    w_g4c = din("w_g4c", [2 * NINP, NZ], F32R)
    w_g5c = din("w_g5c", [2 * NZ, NZ], F32R)
    w_fc1 = din("w_fc1", [NZ, NH], F32R)
    w_disp = din("w_disp", [NH, NINP], F32R)
    w_mu = din("w_mu", [NH, NINP], F32R)
    b_enc1 = din("b_enc1", [128, KE], F32)
    b_enc2p = din("b_enc2p", [128, KE], F32)
    b_enc2r = din("b_enc2r", [1, E], F32)
    b_enc3p = din("b_enc3p", [128, KN], F32)
    b_z = din("b_z", [NZ, 1], F32)
    b_fc1p = din("b_fc1p", [128, KH], F32)
    b_dispp = din("b_dispp", [128, KN], F32)
    b_mup = din("b_mup", [128, KN], F32)
    cTm2 = din("cTm2", [NZ, NC], F32R)     # -2 * cluster.T
    cT = din("cT", [NZ, NC], F32)          # cluster.T

    o_tra1 = dout("o_tra1", [E, SH], F32)
    o_tra2 = dout("o_tra2", [SH, E], F32)
    o_tra3 = dout("o_tra3", [NINP, SH], F32)
    o_z = dout("o_z", [NZ, SH], F32)
    o_h = dout("o_h", [NZ, SH], F32)
    o_pred = dout("o_pred", [NZ, SH], F32)
    o_q = dout("o_q", [NC, SH], F32)
    o_out = dout("o_out", [NINP, SH], F32)
    o_loss = dout("o_loss", [128, 1], F32)

    with tile.TileContext(nc) as tc:
        with (
            tc.tile_pool(name="outer", bufs=1) as outer,
            tc.tile_pool(name="streams", bufs=4) as streams,
            tc.tile_pool(name="ps", bufs=8, space="PSUM") as psum,
            tc.tile_pool(name="dram", bufs=1, space="DRAM") as dram,
        ):
            _emit(nc, tc, outer, streams, psum, dram, locals())

    nc.finalize()
    _NC_CACHE[key] = nc
    return nc


def _emit(nc, tc, outer, streams, psum, dram, T):
    xT, xTb, adjT = T["xT"], T["xTb"], T["adjT"]
    w_enc1, w_enc2, w_enc3, w_z = T["w_enc1"], T["w_enc2"], T["w_enc3"], T["w_z"]
    w_g1, w_g2c, w_g3, w_g4c, w_g5c = (T["w_g1"], T["w_g2c"], T["w_g3"],
                                       T["w_g4c"], T["w_g5c"])
    w_fc1, w_disp, w_mu = T["w_fc1"], T["w_disp"], T["w_mu"]
    o_tra1, o_tra2, o_tra3 = T["o_tra1"], T["o_tra2"], T["o_tra3"]
    o_z, o_h, o_pred, o_q, o_out, o_loss = (T["o_z"], T["o_h"], T["o_pred"],
                                            T["o_q"], T["o_out"], T["o_loss"])

    def ps512(dt=F32):
        return psum.tile([128, 512], dt, tag="ps", name="ps")

    # ---- constants ----
    ones_r1x128 = outer.tile([1, 128], F32, tag="ones128")
    nc.vector.memset(ones_r1x128[:], 1.0)
    ones_c10 = outer.tile([NZ, 1], F32, tag="ones10")
    nc.vector.memset(ones_c10[:], 1.0)
    ones_r1x10 = outer.tile([1, NZ], F32, tag="onesr10")
    nc.vector.memset(ones_r1x10[:], 1.0)
    ones_r1x512 = outer.tile([1, 512], F32, tag="onesr512")
    nc.vector.memset(ones_r1x512[:], 1.0)
    ident = outer.tile([128, 128], F16, tag="ident")
    make_identity(nc, ident[:])
    cb2 = outer.tile([128, 1], BF16, tag="cb2")
    nc.vector.memset(cb2[:], 2.0)
    cbe = outer.tile([128, 1], BF16, tag="cbe")
    nc.vector.memset(cbe[:], EPS)

    # ---- small loads (biases etc.) ----
    def load_small(name, src, shape, dt=F32):
        t = outer.tile(shape, dt, tag=name)
        nc.sync.dma_start(t[:], src[tuple(slice(0, s) for s in shape)])
        return t

    bias1 = load_small("bias1", T["b_enc1"], [128, KE])
    bias2p = load_small("bias2p", T["b_enc2p"], [128, KE])
    bias2r = load_small("bias2r", T["b_enc2r"], [1, E])
    bias3p = load_small("bias3p", T["b_enc3p"], [128, KN])
    biasz = load_small("biasz", T["b_z"], [NZ, 1])
    biasf1 = load_small("biasf1", T["b_fc1p"], [128, KH])
    biasd = load_small("biasd", T["b_dispp"], [128, KN])
    biasm = load_small("biasm", T["b_mup"], [128, KN])
    ctm2 = load_small("ctm2", T["cTm2"], [NZ, NC], F32R)
    ctf = load_small("ctf", T["cT"], [NZ, NC])

    partials = outer.tile([128, 2 * G], F32, tag="partials")

    # small persistent activations (feature-major [10, 1024])
    zT = outer.tile([NZ, SH], F32, tag="zT")
    zr = outer.tile([NZ, SH], F32R, tag="zr")
    hT = outer.tile([NZ, SH], F32, tag="hT")
    relu_h = outer.tile([NZ, SH], F32R, tag="relu_h")
    h_r = outer.tile([NZ, SH], F32R, tag="h_r")
    h5T = outer.tile([NZ, SH], F32, tag="h5T")
    t45 = outer.tile([128, KA, NZ], F16, tag="t45full")

    # AG dram buffers
    ag_in = [dram.tile([SH, E], F16, name=f"agin{i}") for i in range(3)]
    ag_out = [nc.dram_tensor(f"agout{i}", [NODES, E], F16,
                             addr_space="Shared").ap() for i in range(3)]
    ag_in_s = [dram.tile([SH, NZ], F16, name=f"agins{i}") for i in range(2)]
    ag_out_s = [nc.dram_tensor(f"agouts{i}", [NODES, NZ], F16,
                               addr_space="Shared").ap() for i in range(2)]

    RG = [list(range(NCORES))]

    def allgather(src, dst):
        nc.gpsimd.collective_compute(
            "AllGather", OP.bypass, replica_groups=RG,
            ins=[src.opt()], outs=[dst.opt()])

    # =============== region L1: tra1 / tra2 / h1 ===============
    with tc.tile_pool(name="L1", bufs=1) as L1:
        tra1 = L1.tile([128, KE, SH], F32R, tag="tra1")
        tra2T = L1.tile([128, KE, SH], F32R, tag="tra2T")
        tra2n = L1.tile([128, MN, E], F32R, tag="tra2n")
        h1T = L1.tile([128, KE, SH], F32R, tag="h1T")

        # ---------- enc1 + gnn1, fused over the xT stream ----------
        with tc.tile_pool(name="L2", bufs=1) as L2:
            w1 = L2.tile([128, KN, E], F32R, tag="w1res")
            nc.sync.dma_start(
                w1[:], w_enc1.ap().rearrange("(k p) m -> p k m", p=128))
            t1n = L2.tile([128, MN, E], F16, tag="t1n")
            for h in range(NT):   # node half
                pse = [ps512() for _ in range(KE)]
                psg = [ps512() for _ in range(KE)]
                for k in range(KN):
                    xkt = streams.tile([128, 512], F32R, tag="xkt")
                    nc.sync.dma_start(
                        xkt[:], xT[k * 128:(k + 1) * 128,
                                   h * 512:(h + 1) * 512])
                    g1t = streams.tile([128, E], F32R, tag="wstream")
                    nc.sync.dma_start(g1t[:], w_g1[k * 128:(k + 1) * 128, :])
                    st, sp = (k == 0), (k == KN - 1)
                    for m in range(KE):
                        nc.tensor.matmul(pse[m][:],
                                         w1[:, k, m * 128:(m + 1) * 128],
                                         xkt[:], start=st, stop=sp)
                    for m in range(KE):
                        nc.tensor.matmul(psg[m][:],
                                         xkt[:, m * 128:(m + 1) * 128],
                                         g1t[:], start=st, stop=sp)
                for m in range(KE):
                    nc.scalar.activation(tra1[:, m, h * 512:(h + 1) * 512],
                                         pse[m][:], AF.Relu,
                                         bias=bias1[:, m:m + 1])
                    nc.sync.dma_start(
                        o_tra1[m * 128:(m + 1) * 128, h * 512:(h + 1) * 512],
                        tra1[:, m, h * 512:(h + 1) * 512].bitcast(F32))
                for m in range(KE):
                    nc.scalar.activation(t1n[:, h * KE + m, :], psg[m][:],
                                         AF.Copy)
            for m in range(MN):
                nc.sync.dma_start(ag_in[0][m * 128:(m + 1) * 128, :],
                                  t1n[:, m, :])
        allgather(ag_in[0], ag_out[0])
        if STOP_AFTER < 1:
            return

        # ---------- enc2 (both orientations) ----------
        with tc.tile_pool(name="L2b", bufs=1) as L2b:
            w2c = L2b.tile([128, KE, E], F32R, tag="w2c")
            nc.sync.dma_start(
                w2c[:], w_enc2.ap().rearrange("(k p) m -> p k m", p=128))
            for m in range(KE):
                for h in range(NT):
                    ps = ps512()
                    for k in range(KE):
                        nc.tensor.matmul(ps[:],
                                         w2c[:, k, m * 128:(m + 1) * 128],
                                         tra1[:, k, h * 512:(h + 1) * 512],
                                         start=(k == 0), stop=(k == KE - 1))
                    nc.scalar.activation(tra2T[:, m, h * 512:(h + 1) * 512],
                                         ps[:], AF.Relu,
                                         bias=bias2p[:, m:m + 1])
            psn = [ps512() for _ in range(MN)]
            for k in range(KE):
                e2t = streams.tile([128, E], F32R, tag="wstream")
                nc.sync.dma_start(e2t[:], w_enc2[k * 128:(k + 1) * 128, :])
                for m in range(MN):
                    nc.tensor.matmul(psn[m][:],
                                     tra1[:, k, m * 128:(m + 1) * 128],
                                     e2t[:], start=(k == 0), stop=False)
            for m in range(MN):
                nc.tensor.matmul(psn[m][:], ones_r1x128[:], bias2r[:],
                                 start=False, stop=True)
                nc.scalar.activation(tra2n[:, m, :], psn[m][:], AF.Relu)
                nc.sync.dma_start(o_tra2[m * 128:(m + 1) * 128, :],
                                  tra2n[:, m, :].bitcast(F32))

        if STOP_AFTER < 2:
            return
        # ---------- enc3 -> o_tra3 (spilled to DRAM), z ----------
        with tc.tile_pool(name="L2c", bufs=2) as L2c:
            for m in range(KN):
                w3c = L2c.tile([128, KE, 128], F32R, tag="w3c")
                nc.sync.dma_start(
                    w3c[:], w_enc3[:, m * 128:(m + 1) * 128]
                    .rearrange("(k p) m -> p k m", p=128))
                for h in range(NT):
                    ps = ps512()
                    for k in range(KE):
                        nc.tensor.matmul(ps[:], w3c[:, k, :],
                                         tra2T[:, k, h * 512:(h + 1) * 512],
                                         start=(k == 0), stop=(k == KE - 1))
                    ev = L2c.tile([128, 512], F32R, tag="t3ev")
                    nc.scalar.activation(ev[:], ps[:], AF.Relu,
                                         bias=bias3p[:, m:m + 1])
                    nc.sync.dma_start(
                        o_tra3[m * 128:(m + 1) * 128, h * 512:(h + 1) * 512],
                        ev[:].bitcast(F32))
            wzt = L2c.tile([128, KN, NZ], F32R, tag="wzt")
            nc.sync.dma_start(
                wzt[:], w_z.ap().rearrange("(k p) m -> p k m", p=128))
            for h in range(NT):
                ps = ps512()
                for k in range(KN):
                    t3t = streams.tile([128, 512], F32R, tag="t3r")
                    nc.sync.dma_start(
                        t3t[:], o_tra3[k * 128:(k + 1) * 128,
                                       h * 512:(h + 1) * 512].bitcast(F32R))
                    nc.tensor.matmul(ps[:NZ, :], wzt[:, k, :], t3t[:],
                                     start=(k == 0), stop=(k == KN - 1))
                nc.scalar.activation(zT[:, h * 512:(h + 1) * 512], ps[:NZ, :],
                                     AF.Identity, bias=biasz[:, 0:1])
                nc.scalar.activation(zr[:, h * 512:(h + 1) * 512], ps[:NZ, :],
                                     AF.Identity, bias=biasz[:, 0:1])
            nc.sync.dma_start(o_z[:, :], zT[:])

        if STOP_AFTER < 3:
            return
        # ---------- adj pass helper (feature-major output) ----------
        def adj_pass_fmajor(ag_src, out3d, relu):
            ps = [ps512() for _ in range(2 * KE)]
            for ka in range(KA):
                tk = streams.tile([128, E], F16, tag="Tk")
                nc.sync.dma_start(tk[:], ag_src[ka * 128:(ka + 1) * 128, :])
                ah = streams.tile([128, SH], F16, tag="adjk")
                nc.sync.dma_start(ah[:], adjT[ka * 128:(ka + 1) * 128, :])
                st, sp = (ka == 0), (ka == KA - 1)
                for f in range(KE):
                    for h in range(NT):
                        nc.tensor.matmul(ps[f * 2 + h][:],
                                         tk[:, f * 128:(f + 1) * 128],
                                         ah[:, h * 512:(h + 1) * 512],
                                         start=st, stop=sp)
            for f in range(KE):
                for h in range(NT):
                    nc.scalar.activation(out3d[:, f, h * 512:(h + 1) * 512],
                                         ps[f * 2 + h][:],
                                         AF.Relu if relu else AF.Copy,
                                         scale=1.0 / ASCALE)

        # ---------- pass1: h1 = relu(adj @ T1) ----------
        adj_pass_fmajor(ag_out[0], h1T, relu=True)

        if STOP_AFTER < 4:
            return
        # ---------- T2 = [h1; tra1] @ g2c -> AG2 ----------
        with tc.tile_pool(name="L3a", bufs=1) as L3a:
            t2n = L3a.tile([128, MN, E], F16, tag="t2n")
            psn = [ps512() for _ in range(MN)]
            for k in range(2 * KE):
                g2t = streams.tile([128, E], F32R, tag="wstream")
                nc.sync.dma_start(g2t[:], w_g2c[k * 128:(k + 1) * 128, :])
                src = h1T if k < KE else tra1
                kk = k % KE
                for m in range(MN):
                    nc.tensor.matmul(psn[m][:],
                                     src[:, kk, m * 128:(m + 1) * 128],
                                     g2t[:], start=(k == 0),
                                     stop=(k == 2 * KE - 1))
            for m in range(MN):
                nc.scalar.activation(t2n[:, m, :], psn[m][:], AF.Copy)
                nc.sync.dma_start(ag_in[1][m * 128:(m + 1) * 128, :],
                                  t2n[:, m, :])
        allgather(ag_in[1], ag_out[1])

        if STOP_AFTER < 5:
            return
        # ---------- pass2 (node-major) + H3in + AG3 ----------
        with tc.tile_pool(name="L3", bufs=1) as L3:
            h2n = L3.tile([128, MN, E], F32R, tag="h2n")
            ps = [ps512() for _ in range(MN)]
            for ka in range(KA):
                tk = streams.tile([128, E], F16, tag="Tk")
                nc.sync.dma_start(tk[:], ag_out[1][ka * 128:(ka + 1) * 128, :])
                ah = streams.tile([128, SH], F16, tag="adjk")
                nc.sync.dma_start(ah[:], adjT[ka * 128:(ka + 1) * 128, :])
                st, sp = (ka == 0), (ka == KA - 1)
                for m in range(MN):
                    nc.tensor.matmul(ps[m][:], ah[:, m * 128:(m + 1) * 128],
                                     tk[:], start=st, stop=sp)
            for m in range(MN):
                nc.scalar.activation(h2n[:, m, :], ps[m][:],
                                     AF.Relu, scale=1.0 / ASCALE)
            with tc.tile_pool(name="L3b", bufs=2) as L3b:
                for m in range(MN):
                    tmp = L3b.tile([128, E], F32, tag="h3tmp")
                    nc.vector.tensor_scalar_mul(
                        tmp[:], tra2n[:, m, :].bitcast(F32), SIG)
                    h3i = L3b.tile([128, E], F16, tag="h3i")
                    nc.vector.scalar_tensor_tensor(
                        h3i[:], h2n[:, m, :].bitcast(F32), 1.0 - SIG, tmp[:],
                        op0=OP.mult, op1=OP.add)
                    nc.sync.dma_start(ag_in[2][m * 128:(m + 1) * 128, :],
                                      h3i[:])
        allgather(ag_in[2], ag_out[2])

    if STOP_AFTER < 6:
        return
    # ---- lgamma(y+1) early (overlaps the GNN adj passes) ----
    def lgamma_outer(z, out, tp):
        p2 = tp("p2e")
        nc.vector.scalar_tensor_tensor(p2[:], z, 1.0, z,
                                       op0=OP.add, op1=OP.mult)
        lnw = tp("lnwe")
        nc.scalar.activation(lnw[:], z, AF.Ln, bias=cb2[:, 0:1])
        lnp = tp("lnpe")
        nc.scalar.activation(lnp[:], p2[:], AF.Ln)
        rw = tp("rwe")
        nc.scalar.activation(rw[:], lnw[:], AF.Exp, scale=-1.0)
        nc.vector.scalar_tensor_tensor(out, z, 1.5, lnw[:],
                                       op0=OP.add, op1=OP.mult)
        nc.vector.tensor_tensor(out, out, lnp[:], op=OP.subtract)
        nc.vector.scalar_tensor_tensor(out, rw[:], 1.0 / 12.0, out,
                                       op0=OP.mult, op1=OP.add)

    def _etile(name):
        return outer.tile([128, SH], BF16, tag=name, name=name)

    for g in range(G):
        ybe = _etile("ybe")
        nc.sync.dma_start(ybe[:], xTb[g * 128:(g + 1) * 128, :])
        b1e = _etile("b1e")
        nc.vector.tensor_scalar_add(b1e[:], ybe[:], 1.0)
        lbe = _etile("lbe")
        lgamma_outer(b1e[:], lbe[:], _etile)
        nc.vector.tensor_reduce(partials[:, G + g:G + g + 1], lbe[:],
                                axis=AX.X, op=OP.add)

    # =============== pass3 + gnn3 feature + T4 + AG4 ===============
    with tc.tile_pool(name="L4a", bufs=1) as L4a:
        s3T = L4a.tile([128, KE, SH], F32R, tag="s3T")

        def adj_pass_fmajor2(ag_src, out3d, relu):
            ps = [ps512() for _ in range(2 * KE)]
            for ka in range(KA):
                tk = streams.tile([128, E], F16, tag="Tk")
                nc.sync.dma_start(tk[:], ag_src[ka * 128:(ka + 1) * 128, :])
                ah = streams.tile([128, SH], F16, tag="adjk")
                nc.sync.dma_start(ah[:], adjT[ka * 128:(ka + 1) * 128, :])
                st, sp = (ka == 0), (ka == KA - 1)
                for f in range(KE):
                    for h in range(NT):
                        nc.tensor.matmul(ps[f * 2 + h][:],
                                         tk[:, f * 128:(f + 1) * 128],
                                         ah[:, h * 512:(h + 1) * 512],
                                         start=st, stop=sp)
            for f in range(KE):
                for h in range(NT):
                    nc.scalar.activation(out3d[:, f, h * 512:(h + 1) * 512],
                                         ps[f * 2 + h][:],
                                         AF.Relu if relu else AF.Copy,
                                         scale=1.0 / ASCALE)

        adj_pass_fmajor2(ag_out[2], s3T, relu=False)
        h3T = L4a.tile([128, KN, SH], F32R, tag="h3T")
        with tc.tile_pool(name="L4w", bufs=2) as L4w:
            for m in range(KN):
                w3t = L4w.tile([128, KE, 128], F32R, tag="wg3c")
                nc.sync.dma_start(
                    w3t[:], w_g3[:, m * 128:(m + 1) * 128]
                    .rearrange("(k p) m -> p k m", p=128))
                for h in range(NT):
                    ps = ps512()
                    for k in range(KE):
                        nc.tensor.matmul(ps[:], w3t[:, k, :],
                                         s3T[:, k, h * 512:(h + 1) * 512],
                                         start=(k == 0), stop=(k == KE - 1))
                    nc.scalar.activation(h3T[:, m, h * 512:(h + 1) * 512],
                                         ps[:], AF.Relu)
        # T4^T [10, 1024] = g4c.T @ [h3T; tra3]
        with tc.tile_pool(name="L4t", bufs=1) as L4t:
            w4t = L4t.tile([128, 2 * KN, NZ], F32R, tag="w4t")
            nc.sync.dma_start(
                w4t[:], w_g4c.ap().rearrange("(k p) m -> p k m", p=128))
            t4T = L4t.tile([NZ, SH], F16, tag="t4T")
            for h in range(NT):
                ps = ps512()
                for k in range(2 * KN):
                    if k < KN:
                        rhs = h3T[:, k, h * 512:(h + 1) * 512]
                    else:
                        t3t = streams.tile([128, 512], F32R, tag="t3r")
                        nc.sync.dma_start(
                            t3t[:],
                            o_tra3[(k - KN) * 128:(k - KN + 1) * 128,
                                   h * 512:(h + 1) * 512].bitcast(F32R))
                        rhs = t3t[:]
                    nc.tensor.matmul(ps[:NZ, :], w4t[:, k, :], rhs,
                                     start=(k == 0), stop=(k == 2 * KN - 1))
                nc.scalar.activation(t4T[:, h * 512:(h + 1) * 512],
                                     ps[:NZ, :], AF.Copy)
            t4n = L4t.tile([128, MN, NZ], F16, tag="t4n")
            for j in range(MN):
                pst = ps512(F16)
                nc.tensor.transpose(pst[:, :NZ],
                                    t4T[:, j * 128:(j + 1) * 128],
                                    ident[:NZ, :NZ])
                nc.scalar.activation(t4n[:, j, :], pst[:, :NZ], AF.Copy)
                nc.sync.dma_start(ag_in_s[0][j * 128:(j + 1) * 128, :],
                                  t4n[:, j, :])
    allgather(ag_in_s[0], ag_out_s[0])

    if STOP_AFTER < 7:
        return
    # =============== pass4: h = adj @ T4 (no relu on h) ===============
    nc.sync.dma_start(
        t45[:], ag_out_s[0].rearrange("(k p) m -> p k m", p=128))
    ps0, ps1 = ps512(), ps512()
    for ka in range(KA):
        ak = streams.tile([128, SH], F16, tag="adjw")
        nc.sync.dma_start(ak[:], adjT[ka * 128:(ka + 1) * 128, :])
        st, sp = (ka == 0), (ka == KA - 1)
        nc.tensor.matmul(ps0[:NZ, :], t45[:, ka, :], ak[:, :512],
                         start=st, stop=sp)
        nc.tensor.matmul(ps1[:NZ, :], t45[:, ka, :], ak[:, 512:],
                         start=st, stop=sp)
    for h, ps in ((0, ps0), (1, ps1)):
        sl = slice(h * 512, (h + 1) * 512)
        nc.scalar.activation(hT[:, sl], ps[:NZ, :], AF.Copy, scale=1.0 / ASCALE)
        nc.scalar.activation(relu_h[:, sl], ps[:NZ, :], AF.Relu,
                             scale=1.0 / ASCALE)
        nc.scalar.activation(h_r[:, sl], ps[:NZ, :], AF.Copy,
                             scale=1.0 / ASCALE)
    nc.sync.dma_start(o_h[:, :], hT[:])

    if STOP_AFTER < 8:
        return
    # =============== T5 + AG5 + pass5 ===============
    with tc.tile_pool(name="L5", bufs=1) as L5:
        w5a = L5.tile([NZ, NZ], F32R, tag="w5a")
        nc.sync.dma_start(w5a[:], w_g5c[0:NZ, :])
        w5b = L5.tile([NZ, NZ], F32R, tag="w5b")
        nc.sync.dma_start(w5b[:], w_g5c[NZ:2 * NZ, :])
        t5T = L5.tile([NZ, SH], F16, tag="t5T")
        for h in range(NT):
            sl = slice(h * 512, (h + 1) * 512)
            ps = ps512()
            nc.tensor.matmul(ps[:NZ, :], w5a[:], relu_h[:, sl],
                             start=True, stop=False)
            nc.tensor.matmul(ps[:NZ, :], w5b[:], zr[:, sl],
                             start=False, stop=True)
            nc.scalar.activation(t5T[:, h * 512:(h + 1) * 512],
                                 ps[:NZ, :], AF.Copy)
        t5n = L5.tile([128, MN, NZ], F16, tag="t5n")
        for j in range(MN):
            pst = ps512(F16)
            nc.tensor.transpose(pst[:, :NZ], t5T[:, j * 128:(j + 1) * 128],
                                ident[:NZ, :NZ])
            nc.scalar.activation(t5n[:, j, :], pst[:, :NZ], AF.Copy)
            nc.sync.dma_start(ag_in_s[1][j * 128:(j + 1) * 128, :],
                              t5n[:, j, :])
    allgather(ag_in_s[1], ag_out_s[1])
    nc.sync.dma_start(
        t45[:], ag_out_s[1].rearrange("(k p) m -> p k m", p=128))
    ps0, ps1 = ps512(), ps512()
    for ka in range(KA):
        ak = streams.tile([128, SH], F16, tag="adjw")
        nc.sync.dma_start(ak[:], adjT[ka * 128:(ka + 1) * 128, :])
        st, sp = (ka == 0), (ka == KA - 1)
        nc.tensor.matmul(ps0[:NZ, :], t45[:, ka, :], ak[:, :512],
                         start=st, stop=sp)
        nc.tensor.matmul(ps1[:NZ, :], t45[:, ka, :], ak[:, 512:],
                         start=st, stop=sp)
    for h, ps in ((0, ps0), (1, ps1)):
        nc.scalar.activation(h5T[:, h * 512:(h + 1) * 512], ps[:NZ, :],
                             AF.Copy, scale=1.0 / ASCALE)

    if STOP_AFTER < 9:
        return
    # =============== predict = softmax(h5), q (student-t) ===============
    with tc.tile_pool(name="L6", bufs=1) as L6:
        def rowsum_bcast_mult(src, dst):
            sm = L6.tile([1, SH], F32, tag="sm")
            for h in range(NT):
                ps = ps512()
                nc.tensor.matmul(ps[:1, :], ones_c10[:],
                                 src[:, h * 512:(h + 1) * 512],
                                 start=True, stop=True)
                nc.scalar.activation(sm[:, h * 512:(h + 1) * 512],
                                     ps[:1, :], AF.Copy)
            rs = L6.tile([1, SH], F32, tag="rs")
            nc.vector.reciprocal(rs[:], sm[:])
            bc = L6.tile([NZ, SH], F32, tag="bc")
            for h in range(NT):
                ps = ps512()
                nc.tensor.matmul(ps[:NZ, :], ones_r1x10[:],
                                 rs[:, h * 512:(h + 1) * 512],
                                 start=True, stop=True)
                nc.scalar.activation(bc[:, h * 512:(h + 1) * 512],
                                     ps[:NZ, :], AF.Copy)
            nc.vector.tensor_tensor(dst[:], src[:], bc[:], op=OP.mult)

        ex = L6.tile([NZ, SH], F32, tag="ex")
        nc.scalar.activation(ex[:], h5T[:], AF.Exp)
        pred = L6.tile([NZ, SH], F32, tag="pred")
        rowsum_bcast_mult(ex, pred)
        nc.sync.dma_start(o_pred[:, :], pred[:])

        hsq = L6.tile([NZ, SH], F32, tag="hsq")
        nc.scalar.activation(hsq[:], hT[:], AF.Square)
        hn2 = L6.tile([1, SH], F32, tag="hn2")
        for h in range(NT):
            ps = ps512()
            nc.tensor.matmul(ps[:1, :], ones_c10[:],
                             hsq[:, h * 512:(h + 1) * 512],
                             start=True, stop=True)
            nc.scalar.activation(hn2[:, h * 512:(h + 1) * 512],
                                 ps[:1, :], AF.Copy)
        csq = L6.tile([NZ, NC], F32, tag="csq")
        nc.scalar.activation(csq[:], ctf[:], AF.Square)
        pcs = ps512()
        nc.tensor.matmul(pcs[:1, :NC], ones_c10[:], csq[:],
                         start=True, stop=True)
        cn2 = L6.tile([1, NC], F32, tag="cn2")
        nc.scalar.activation(cn2[:], pcs[:1, :NC], AF.Copy)
        qu = L6.tile([NC, SH], F32, tag="qu")
        for h in range(NT):
            sl = slice(h * 512, (h + 1) * 512)
            ps = ps512()
            nc.tensor.matmul(ps[:NC, :], ctm2[:], h_r[:, sl],
                             start=True, stop=False)
            nc.tensor.matmul(ps[:NC, :], cn2[:], ones_r1x512[:],
                             start=False, stop=False)
            nc.tensor.matmul(ps[:NC, :], ones_r1x10[:], hn2[:, sl],
                             start=False, stop=True)
            dpo = L6.tile([NC, 512], F32, tag="dpo")
            nc.scalar.activation(dpo[:], ps[:NC, :], AF.Copy, bias=1.0)
            nc.vector.reciprocal(qu[:, sl], dpo[:])
        qn = L6.tile([NC, SH], F32, tag="qn")
        rowsum_bcast_mult(qu, qn)
        nc.sync.dma_start(o_q[:, :], qn[:])

    if STOP_AFTER < 10:
        return
    # =============== NB head + loss ===============
    with tc.tile_pool(name="L7", bufs=1) as L7:
        deco = L7.tile([128, KH, SH], F32R, tag="deco")
        wf = L7.tile([NZ, NH], F32R, tag="wf")
        nc.sync.dma_start(wf[:], w_fc1[:, :])
        for m in range(KH):
            for h in range(NT):
                ps = ps512()
                nc.tensor.matmul(ps[:], wf[:, m * 128:(m + 1) * 128],
                                 relu_h[:, h * 512:(h + 1) * 512],
                                 start=True, stop=True)
                nc.scalar.activation(deco[:, m, h * 512:(h + 1) * 512],
                                     ps[:], AF.Relu, bias=biasf1[:, m:m + 1])

        if STOP_AFTER < 11:
            return
        with (
            tc.tile_pool(name="L7b", bufs=2) as L7b,
            tc.tile_pool(name="L7c", bufs=1) as L7c,
        ):

            def bt2(name):
                return L7b.tile([128, SH], BF16, tag=name, name=name)

            def bt1(name):
                return L7c.tile([128, SH], BF16, tag=name, name=name)

            def lgamma_approx(z, out):
                """L'(z) = (z+1.5)ln(z+2) - ln(z(z+1)) + 1/(12(z+2))."""
                p2 = bt1("p2")
                nc.vector.scalar_tensor_tensor(p2[:], z, 1.0, z,
                                               op0=OP.add, op1=OP.mult)
                lnw = bt1("lnw")
                nc.scalar.activation(lnw[:], z, AF.Ln, bias=cb2[:, 0:1])
                lnp = bt1("lnp")
                nc.scalar.activation(lnp[:], p2[:], AF.Ln)
                rw = bt1("rw")
                nc.scalar.activation(rw[:], lnw[:], AF.Exp, scale=-1.0)
                nc.vector.scalar_tensor_tensor(out, z, 1.5, lnw[:],
                                               op0=OP.add, op1=OP.mult)
                nc.vector.tensor_tensor(out, out, lnp[:], op=OP.subtract)
                nc.vector.scalar_tensor_tensor(out, rw[:], 1.0 / 12.0, out,
                                               op0=OP.mult, op1=OP.add)

            for g in range(G):
                wdc = L7b.tile([128, KH, 128], F32R, tag="wdc")
                nc.sync.dma_start(
                    wdc[:], w_disp[:, g * 128:(g + 1) * 128]
                    .rearrange("(k p) m -> p k m", p=128))
                wmc = L7b.tile([128, KH, 128], F32R, tag="wmc")
                nc.sync.dma_start(
                    wmc[:], w_mu[:, g * 128:(g + 1) * 128]
                    .rearrange("(k p) m -> p k m", p=128))
                th = bt2("th")
                yh = bt2("yh")
                for h in range(NT):
                    sl = slice(h * 512, (h + 1) * 512)
                    psd = ps512()
                    for k in range(KH):
                        nc.tensor.matmul(psd[:], wdc[:, k, :],
                                         deco[:, k, sl],
                                         start=(k == 0), stop=(k == KH - 1))
                    eu = L7b.tile([128, 512], BF16, tag="eu")
                    nc.scalar.activation(eu[:], psd[:], AF.Exp,
                                         bias=biasd[:, g:g + 1])
                    nc.scalar.activation(th[:, sl], eu[:], AF.Ln, bias=1.0)
                    psm = ps512()
                    for k in range(KH):
                        nc.tensor.matmul(psm[:], wmc[:, k, :],
                                         deco[:, k, sl],
                                         start=(k == 0), stop=(k == KH - 1))
                    ot = L7b.tile([128, 512], F32, tag="ot")
                    nc.scalar.activation(ot[:], psm[:], AF.Relu,
                                         bias=biasm[:, g:g + 1])
                    nc.sync.dma_start(o_out[g * 128:(g + 1) * 128, sl], ot[:])
                    nc.vector.tensor_copy(yh[:, sl], ot[:])
                if STOP_AFTER < 12:
                    continue
                # ---- loss terms for this gene tile ----
                yb = bt2("yb")
                nc.sync.dma_start(yb[:], xTb[g * 128:(g + 1) * 128, :])
                la = bt1("la")
                lgamma_approx(th[:], la[:])
                cz = bt1("cz")
                nc.vector.tensor_tensor(cz[:], yb[:], th[:], op=OP.add)
                lc = bt1("lc")
                lgamma_approx(cz[:], lc[:])
                d1 = bt1("d1")
                nc.vector.tensor_tensor(d1[:], th[:], yh[:], op=OP.add)
                ln1 = bt1("ln1")
                nc.scalar.activation(ln1[:], d1[:], AF.Ln)
                lnth = bt1("lnth")
                nc.scalar.activation(lnth[:], th[:], AF.Ln)
                lnyh = bt1("lnyh")
                nc.scalar.activation(lnyh[:], yh[:], AF.Ln, bias=cbe[:, 0:1])
                m1 = bt1("m1")
                nc.vector.tensor_tensor(m1[:], th[:], yb[:], op=OP.add)
                nc.vector.tensor_tensor(m1[:], m1[:], ln1[:], op=OP.mult)
                m2 = bt1("m2")
                nc.vector.tensor_tensor(m2[:], th[:], lnth[:], op=OP.mult)
                m3 = bt1("m3")
                nc.vector.tensor_tensor(m3[:], yb[:], lnyh[:], op=OP.mult)
                nc.vector.tensor_tensor(m1[:], m1[:], m2[:], op=OP.subtract)
                nc.vector.tensor_tensor(m1[:], m1[:], m3[:], op=OP.subtract)
                nc.vector.tensor_tensor(la[:], la[:], lc[:], op=OP.subtract)
                nc.vector.tensor_tensor(la[:], la[:], m1[:], op=OP.add)
                nc.vector.tensor_reduce(partials[:, g:g + 1], la[:],
                                        axis=AX.X, op=OP.add)

    if STOP_AFTER < 12:
        return
    rsum = outer.tile([128, 1], F32, tag="rsum")
    nc.vector.tensor_reduce(rsum[:], partials[:], axis=AX.X, op=OP.add)
    nc.sync.dma_start(o_loss[:, :], rsum[:])


def _prep_inputs(x, adj, p):
    """Host-side sharding / layout prep. Returns per-core input maps."""
    x = np.asarray(x, np.float32)
    adj = np.asarray(adj, np.float32)

    def pad_r(w, r, c):
        return _r32r(_pad2(w, r, c))

    shared = {
        "w_enc1": pad_r(p["enc1_w"], NINP, E),
        "w_enc2": pad_r(p["enc2_w"], E, E),
        "w_enc3": pad_r(p["enc3_w"], E, NINP),
        "w_z": pad_r(p["z_w"], NINP, NZ),
        "w_g1": pad_r(p["gnn1_w"], NINP, E),
        "w_g2c": _r32r(np.concatenate([
            _pad2((1 - SIG) * np.asarray(p["gnn2_w"]), E, E),
            _pad2(SIG * np.asarray(p["gnn2_w"]), E, E)])),
        "w_g3": pad_r(p["gnn3_w"], E, NINP),
        "w_g4c": _r32r(np.concatenate([
            _pad2((1 - SIG) * np.asarray(p["gnn4_w"]), NINP, NZ),
            _pad2(SIG * np.asarray(p["gnn4_w"]), NINP, NZ)])),
        "w_g5c": _r32r(np.concatenate([
            (1 - SIG) * np.asarray(p["gnn5_w"], np.float32),
            SIG * np.asarray(p["gnn5_w"], np.float32)])),
        "w_fc1": pad_r(p["fc1_w"], NZ, NH),
        "w_disp": pad_r(p["disp_w"], NH, NINP),
        "w_mu": pad_r(p["mu_w"], NH, NINP),
        "cTm2": _r32r(np.ascontiguousarray(
            -2.0 * np.asarray(p["cluster"], np.float32).T)),
        "cT": np.ascontiguousarray(np.asarray(p["cluster"], np.float32).T),
    }

    def bias_p(b, padded, kt):
        v = np.zeros(padded, np.float32)
        b = np.asarray(b, np.float32)
        v[:b.shape[0]] = b
        return np.ascontiguousarray(v.reshape(kt, 128).T)

    shared["b_enc1"] = bias_p(p["enc1_b"], E, KE)
    shared["b_enc2p"] = bias_p(p["enc2_b"], E, KE)
    shared["b_enc2r"] = _pad2(
        np.asarray(p["enc2_b"], np.float32)[None, :], 1, E)
    shared["b_enc3p"] = bias_p(p["enc3_b"], NINP, KN)
    shared["b_z"] = np.ascontiguousarray(
        np.asarray(p["z_b"], np.float32).reshape(NZ, 1))
    shared["b_fc1p"] = bias_p(p["fc1_b"], NH, KH)
    shared["b_dispp"] = bias_p(p["disp_b"], NINP, KN)
    shared["b_mup"] = bias_p(p["mu_b"], NINP, KN)

    xp = np.zeros((NODES, NINP), np.float32)
    xp[:, :NIN] = x
    in_maps = []
    for c in range(NCORES):
        rows = slice(c * SH, (c + 1) * SH)
        xTc = np.ascontiguousarray(xp[rows].T)
        m = dict(shared)
        m["xT"] = _r32r(xTc)
        m["xTb"] = xTc.astype(ml_dtypes.bfloat16)
        m["adjT"] = np.ascontiguousarray(
            (adj[rows] * ASCALE).T.astype(np.float16))
        in_maps.append(m)
    return in_maps


def run_device(x, adj, params, trace=False):
    nc = build()
    in_maps = _prep_inputs(x, adj, params)
    return run_bass_kernel_spmd(nc, in_maps, list(range(NCORES)), trace=trace)


def assemble(results):
    """Gather per-core outputs into full reference-shaped arrays."""
    def cat_T(name, cols):  # feature-major [F, SH] -> [8192, cols]
        return np.ascontiguousarray(np.concatenate(
            [r[name].T for r in results], axis=0)[:, :cols]).astype(np.float32)

    output = cat_T("o_out", NIN)
    tra1 = cat_T("o_tra1", ET)
    tra3 = cat_T("o_tra3", NIN)
    tra2 = np.ascontiguousarray(np.concatenate(
        [r["o_tra2"] for r in results], axis=0)[:, :ET])
    z = cat_T("o_z", NZ)
    h = cat_T("o_h", NZ)
    predict = cat_T("o_pred", NZ)
    q = cat_T("o_q", NC)
    total = sum(float(r["o_loss"].sum()) for r in results)
    n_tot = NCORES * NINP * SH
    n_real = NODES * NIN
    loss = np.float32((total + n_tot * (C_STIRLING - 3.0)) / n_real)
    return (output, q, predict, z, np.float32(0.0), loss,
            (h, tra1, tra2, tra3))


def kernel(x, adj, params):
    res = run_device(x, adj, params, trace=False)
    return assemble(res.results)
